# revision 1
# baseline (speedup 1.0000x reference)
"""Trainium2 Bass kernel for nn_Attention_52982716563627.

Module: qkv = x@W_atten + b_atten; per-head scores s = q k^T (no scaling);
mask applied as w*mask + (1-mask)*1e5; softmax over the HEAD axis (quirk!);
a = p @ v; out = a @ W_proj + b_proj.

Key identity: softmax is over heads at each (i,j). Masked entries (+1e5 for
all 16 heads) softmax to exactly 1/16. Scores are bounded (|s| < ~40), so
max-subtraction can be skipped: set masked scores to 0 -> exp=1 -> uniform
1/16 falls out of the normalization automatically:
    e = exp(s * causal_mask);  Z = sum_h e;  p = e / Z;  a = p @ v
which matches the reference exactly (softmax is shift-invariant).

Sharding: 8 cores = 2 batches x 4 query-blocks of 512 rows (SPMD, one
program; per-core differences enter only via input data: xTq slice + masks).
K/V are computed on every core of a batch group from the full x[b]; v is
bounced through DRAM to fit SBUF.

Layouts (per core):
  kT resident [128p=(h%2)*64+d, 8=h//2, 2048 keys]     (head-major cols)
  qT resident [128, 8, 512 own q rows]
  v in DRAM   [2048 keys, 1024 cols head-major]
  scores computed transposed: sT[key, q] so AV needs no transposes; K=64
  score matmuls on even/odd partition halves pack 2x via PE 64x128 tiling.
  e tiles [128 keys, 16 h, 256 q]; Z = chain-sum over h; p = e * (1/Z).
  AV accumulates over 16 key chunks into 4 PSUM banks (4 heads per bank:
  2 on partition halves x 2 on free halves); each bank is one accumulation
  group opened by a zeroing K=1 matmul (start=True wipes the whole 2KB
  zero-region, so only one start per bank is allowed).
"""

import numpy as np
import ml_dtypes

import concourse.bass as bass
import concourse.bacc as bacc
import concourse.mybir as mybir
import concourse.tile as tile
from concourse import bass_utils

N_CORES = 8
B, S, E = 2, 2048, 1024
H, HD = 16, 64
DQ = 512   # q rows per core
QH = 256   # q tile (half block)
KC = 128   # key chunk
NKC = S // KC  # 16
FP = mybir.dt.float32
FR = mybir.dt.float32r
BF = mybir.dt.bfloat16
AF = mybir.ActivationFunctionType

# Tuning flags (empirically validated on HW; see test logs)
CFG = dict(
    r_qkv=False,    # float32r for QKV + proj matmuls
    r_s=False,      # float32r for score matmuls
    bf16_sm=False,  # bf16 e/p tiles, post-exp mask, broadcast normalize
    bf16_sm2=False, # bf16 e/p tiles, per-head ops (no broadcast APs)
    recip_fast=False,
)


def _bcast_mid(ap, n):
    """View a [128, Q] AP as [128, n, Q] with 0-stride middle dim."""
    return bass.AP(tensor=ap.tensor, offset=ap.offset,
                   ap=[ap.ap[0], [0, n], ap.ap[1]])


def build_program(reps: int = 1, debug_taps: bool = False, cfg=None):
    cfg = dict(CFG, **(cfg or {}))
    nc = bacc.Bacc("TRN2", target_bir_lowering=False, debug=False,
                   num_devices=N_CORES)
    # Matmul-operand dtypes. float32r = fp32 storage that the PE consumes at
    # 1 cycle/row (vs 4 for fp32); walrus requires operands to be *declared*
    # (and rounded by their producers) as float32r.
    MQ = FR if cfg["r_qkv"] else FP        # QKV + proj operands
    MS = FR if cfg["r_s"] else FP          # score operands (kT, qT)
    # e/p/v dtype: fp32r AV is ISA-invalid with column-tiled outputs, so the
    # non-bf16 path keeps AV in plain fp32.
    anybf = cfg["bf16_sm"] or cfg["bf16_sm2"]
    EDT = BF if anybf else FP

    def rq(ap):
        return ap

    def rs(ap):
        return ap

    if debug_taps:
        dbg_eb = nc.dram_tensor("dbg_eb", [128, H, QH], FP,
                                kind="ExternalOutput")

    xT_d = nc.dram_tensor("xT", [E, S], MQ, kind="ExternalInput")
    xTq_d = nc.dram_tensor("xTq", [E, DQ], MQ, kind="ExternalInput")
    wq_d = nc.dram_tensor("wq", [E, E], MQ, kind="ExternalInput")
    wk_d = nc.dram_tensor("wk", [E, E], MQ, kind="ExternalInput")
    wv_d = nc.dram_tensor("wv", [E, E], MQ, kind="ExternalInput")
    wo_d = nc.dram_tensor("wo", [E, E], MQ, kind="ExternalInput")
    bq_d = nc.dram_tensor("bq", [128, 8], FP, kind="ExternalInput")
    bk_d = nc.dram_tensor("bk", [128, 8], FP, kind="ExternalInput")
    bv_d = nc.dram_tensor("bv", [1, E], MQ, kind="ExternalInput")
    bo_d = nc.dram_tensor("bo", [1, E], MQ, kind="ExternalInput")
    onesr_d = nc.dram_tensor("onesr", [1, 128], MQ, kind="ExternalInput")
    zerosr_d = nc.dram_tensor("zerosr", [1, 512], MQ, kind="ExternalInput")
    mdt = BF if cfg["bf16_sm"] else FP  # mask dtype (sm2 masks pre-exp in fp32)
    mask_d = nc.dram_tensor("maskT", [NKC, KC, DQ], mdt, kind="ExternalInput")
    mask1_d = nc.dram_tensor("mask1T", [NKC, KC, DQ], mdt,
                             kind="ExternalInput")
    out_d = nc.dram_tensor("out", [DQ, E], FP, kind="ExternalOutput")

    with tile.TileContext(nc) as tc:
        with (
            tc.tile_pool(name="consts", bufs=1) as consts,
            tc.tile_pool(name="kt", bufs=1) as ktp,
            tc.tile_pool(name="qt", bufs=1) as qtp,
            tc.tile_pool(name="vdram", bufs=1, space="DRAM") as vdp,
        ):
            ones_sb = consts.tile([1, 128], MQ)
            zeros_sb = consts.tile([1, 512], MQ)
            bq_sb = consts.tile([128, 8], FP)
            bk_sb = consts.tile([128, 8], FP)
            bv_sb = consts.tile([1, E], MQ)
            bo_sb = consts.tile([1, E], MQ)
            nc.sync.dma_start(bq_sb[:], bq_d[:])
            nc.sync.dma_start(bk_sb[:], bk_d[:])
            nc.sync.dma_start(ones_sb[:], onesr_d[:])
            nc.sync.dma_start(zeros_sb[:], zerosr_d[:])
            nc.sync.dma_start(bv_sb[:], bv_d[:])
            nc.sync.dma_start(bo_sb[:], bo_d[:])

            kt = ktp.tile([128, 8, S], MS)
            qt = qtp.tile([128, 8, DQ], MS)
            v_dram = vdp.tile([S, E], EDT)

            loop = tc.For_i(0, reps, 1) if reps > 1 else None
            if loop is not None:
                loop.__enter__()

            # ---------------- Phase A: qT, v, kT ----------------
            with (
                tc.tile_pool(name="xt", bufs=1) as xtp,
                tc.tile_pool(name="xtq", bufs=1) as xtqp,
                tc.tile_pool(name="wstr", bufs=2) as wstr,
                tc.tile_pool(name="wvp", bufs=1) as wvp,
                tc.tile_pool(name="vout", bufs=2) as voutp,
                tc.tile_pool(name="pskq", bufs=2, space="PSUM") as pskq,
                tc.tile_pool(name="psv", bufs=2, space="PSUM") as psv,
            ):
                xt = xtp.tile([128, 8, S], MQ)
                for e in range(8):
                    nc.sync.dma_start(
                        xt[:, e, :], xT_d[e * 128:(e + 1) * 128, :])
                xtq = xtqp.tile([128, 8, DQ], MQ)
                for e in range(8):
                    nc.sync.dma_start(
                        xtq[:, e, :], xTq_d[e * 128:(e + 1) * 128, :])

                # qT: out.T orientation [cols, rows]; bias per-partition
                for ct in range(8):
                    wct = wstr.tile([128, 8, 128], MQ, tag="wct")
                    nc.sync.dma_start(
                        wct[:],
                        wq_d[:, ct * 128:(ct + 1) * 128].rearrange(
                            "(e p) c -> p e c", p=128))
                    ps = pskq.tile([128, DQ], FP)
                    for e in range(8):
                        nc.tensor.matmul(ps[:], rq(wct[:, e, :]),
                                         rq(xtq[:, e, :]),
                                         start=(e == 0), stop=(e == 7))
                    nc.scalar.activation(qt[:, ct, :], ps[:], AF.Identity,
                                         bias=bq_sb[:, ct:ct + 1])

                # v natural [keys, cols]: bias via K=1 ones matmul
                for cc in range(2):
                    wvcc = wvp.tile([128, 8, 512], MQ, tag="wvcc")
                    nc.sync.dma_start(
                        wvcc[:],
                        wv_d[:, cc * 512:(cc + 1) * 512].rearrange(
                            "(e p) c -> p e c", p=128))
                    for rt in range(16):
                        ps = psv.tile([128, 512], FP)
                        for e in range(8):
                            nc.tensor.matmul(
                                ps[:], rq(xt[:, e, rt * 128:(rt + 1) * 128]),
                                rq(wvcc[:, e, :]), start=(e == 0), stop=False)
                        nc.tensor.matmul(ps[:], ones_sb[:1, :],
                                         bv_sb[:1, cc * 512:(cc + 1) * 512],
                                         start=False, stop=True)
                        vo = voutp.tile([128, 512], EDT, tag="vo")
                        nc.scalar.activation(vo[:], ps[:], AF.Copy)
                        nc.sync.dma_start(
                            v_dram[rt * 128:(rt + 1) * 128,
                                   cc * 512:(cc + 1) * 512], vo[:])

                # kT (xt still resident)
                for ct in range(8):
                    wct = wstr.tile([128, 8, 128], MQ, tag="wct")
                    nc.sync.dma_start(
                        wct[:],
                        wk_d[:, ct * 128:(ct + 1) * 128].rearrange(
                            "(e p) c -> p e c", p=128))
                    for kc4 in range(4):
                        ps = pskq.tile([128, DQ], FP)
                        for e in range(8):
                            nc.tensor.matmul(
                                ps[:], rq(wct[:, e, :]),
                                rq(xt[:, e, kc4 * 512:(kc4 + 1) * 512]),
                                start=(e == 0), stop=(e == 7))
                        nc.scalar.activation(
                            kt[:, ct, kc4 * 512:(kc4 + 1) * 512], ps[:],
                            AF.Identity, bias=bk_sb[:, ct:ct + 1])

            # ---------------- Phase B: attention + proj ----------------
            with (
                tc.tile_pool(name="ebig", bufs=2) as ebp,
                tc.tile_pool(name="zr", bufs=3) as zrp,
                tc.tile_pool(name="mstr", bufs=3) as mstr,
                tc.tile_pool(name="vstr", bufs=3) as vstr,
                tc.tile_pool(name="at", bufs=2) as atp,
                tc.tile_pool(name="wop", bufs=2) as wop,
                tc.tile_pool(name="outp", bufs=2) as outp,
                tc.tile_pool(name="pss", bufs=3, space="PSUM") as pss,
                tc.tile_pool(name="psav", bufs=1, space="PSUM") as psav,
                tc.tile_pool(name="pspj", bufs=1, space="PSUM") as pspj,
            ):
                for qh in range(2):
                    av = [psav.tile([128, 512], FP, tag=f"av{g}",
                                    name=f"av{g}")
                          for g in range(4)]
                    for g in range(4):
                        nc.tensor.matmul(av[g][:], ones_sb[:1, :],
                                         zeros_sb[:1, :], start=True,
                                         stop=False, skip_group_check=True)
                    for c in range(NKC):
                        mt = mstr.tile([128, QH], mdt, tag="mt")
                        nc.sync.dma_start(
                            mt[:], mask_d[c, :, qh * QH:(qh + 1) * QH])
                        if cfg["bf16_sm"]:
                            m1 = mstr.tile([128, QH], mdt, tag="m1")
                            nc.sync.dma_start(
                                m1[:], mask1_d[c, :, qh * QH:(qh + 1) * QH])
                        vch = vstr.tile([128, E], EDT, tag="vch")
                        nc.sync.dma_start(
                            vch[:], v_dram[c * 128:(c + 1) * 128, :])
                        eb = ebp.tile([128, H, QH], EDT, tag="eb")
                        for h in range(H):
                            po = (h % 2) * 64
                            ps = pss.tile([128, QH], FP, tag="ps")
                            nc.tensor.matmul(
                                ps[:],
                                rs(kt[po:po + 64, h // 2,
                                      c * 128:(c + 1) * 128]),
                                rs(qt[po:po + 64, h // 2,
                                      qh * QH:(qh + 1) * QH]),
                                start=True, stop=True)
                            if not cfg["bf16_sm"]:
                                nc.vector.tensor_mul(ps[:], ps[:], mt[:])
                            nc.scalar.activation(eb[:, h, :], ps[:], AF.Exp)
                        if cfg["bf16_sm"]:
                            # e <- e*m + (1-m), batched over heads
                            nc.vector.tensor_mul(eb[:], eb[:],
                                                 _bcast_mid(mt[:], H))
                            nc.vector.tensor_add(eb[:], eb[:],
                                                 _bcast_mid(m1[:], H))
                        # Z = sum over heads (chained adds), R = 1/Z
                        z = zrp.tile([128, QH], EDT, tag="z")
                        nc.vector.tensor_add(z[:], eb[:, 0, :], eb[:, 1, :])
                        for h in range(2, H):
                            nc.vector.tensor_add(z[:], z[:], eb[:, h, :])
                        zf = zrp.tile([128, QH], FP, tag="zf")
                        if anybf:
                            nc.vector.tensor_copy(zf[:], z[:])
                        else:
                            zf = z
                        r = zrp.tile([128, QH], FP, tag="r")
                        if cfg["recip_fast"]:
                            nc.vector.reciprocal_approx_fast(r[:], zf[:])
                        else:
                            nc.vector.reciprocal(r[:], zf[:])
                        if cfg["bf16_sm"]:
                            rb = zrp.tile([128, QH], BF, tag="rb")
                            nc.vector.tensor_copy(rb[:], r[:])
                            nc.vector.tensor_mul(eb[:], eb[:],
                                                 _bcast_mid(rb[:], H))
                        elif cfg["bf16_sm2"]:
                            rb = zrp.tile([128, QH], BF, tag="rb")
                            nc.vector.tensor_copy(rb[:], r[:])
                            for h in range(H):
                                nc.vector.tensor_mul(eb[:, h, :],
                                                     eb[:, h, :], rb[:])
                        else:
                            for h in range(H):
                                nc.vector.tensor_mul(eb[:, h, :],
                                                     eb[:, h, :], r[:])
                        if debug_taps and qh == 0 and c == 2:
                            nc.sync.dma_start(dbg_eb[:], eb[:])
                        for h in range(H):
                            g, g2, po = h // 4, (h // 2) % 2, (h % 2) * 64
                            nc.tensor.matmul(
                                av[g][po:po + 64, g2 * QH:(g2 + 1) * QH],
                                vch[:, h * 64:(h + 1) * 64], eb[:, h, :],
                                start=False, stop=False,
                                skip_group_check=True)
                    for g in range(4):
                        nc.tensor.matmul(av[g][:], ones_sb[:1, :],
                                         zeros_sb[:1, :], start=False,
                                         stop=True, skip_group_check=True)
                    # aT and projection for this q-half
                    at = atp.tile([128, 8, QH], MQ, tag="at")
                    for j in range(8):
                        nc.scalar.activation(
                            at[:, j, :],
                            av[j // 2][:, (j % 2) * QH:(j % 2 + 1) * QH],
                            AF.Copy)
                    for cc in range(2):
                        wocc = wop.tile([128, 8, 512], MQ, tag="wocc")
                        nc.sync.dma_start(
                            wocc[:],
                            wo_d[:, cc * 512:(cc + 1) * 512].rearrange(
                                "(e p) c -> p e c", p=128))
                        for qs in range(2):
                            ps = pspj.tile([128, 512], FP, tag="pj")
                            for e in range(8):
                                nc.tensor.matmul(
                                    ps[:],
                                    rq(at[:, e, qs * 128:(qs + 1) * 128]),
                                    rq(wocc[:, e, :]),
                                    start=(e == 0), stop=False)
                            nc.tensor.matmul(
                                ps[:], ones_sb[:1, :],
                                bo_sb[:1, cc * 512:(cc + 1) * 512],
                                start=False, stop=True)
                            ot = outp.tile([128, 512], FP, tag="ot")
                            nc.scalar.activation(ot[:], ps[:], AF.Copy)
                            nc.sync.dma_start(
                                out_d[qh * QH + qs * 128:
                                      qh * QH + (qs + 1) * 128,
                                      cc * 512:(cc + 1) * 512], ot[:])

            if loop is not None:
                loop.__exit__(None, None, None)

    nc.compile()
    return nc


def prep_inputs(x, W_atten, b_atten, W_proj, b_proj, cfg=None):
    """Host-side prep: per-core input dicts (numpy)."""
    cfg = dict(CFG, **(cfg or {}))
    x = np.asarray(x, dtype=np.float32)
    W3 = np.asarray(W_atten, dtype=np.float32).reshape(E, H, 3, HD)
    b3 = np.asarray(b_atten, dtype=np.float32).reshape(H, 3, HD)
    wq = np.ascontiguousarray(W3[:, :, 0, :].reshape(E, E))
    wk = np.ascontiguousarray(W3[:, :, 1, :].reshape(E, E))
    wv = np.ascontiguousarray(W3[:, :, 2, :].reshape(E, E))
    bq = np.ascontiguousarray(b3[:, 0, :].reshape(E).reshape(8, 128).T)
    bk = np.ascontiguousarray(b3[:, 1, :].reshape(E).reshape(8, 128).T)
    bv = b3[:, 2, :].reshape(1, E).copy()
    wo = np.asarray(W_proj, dtype=np.float32)
    bo = np.asarray(b_proj, dtype=np.float32).reshape(1, E).copy()
    mdt = ml_dtypes.bfloat16 if cfg["bf16_sm"] else np.float32

    in_maps = []
    for core in range(N_CORES):
        b, qb = core // 4, core % 4
        xT = np.ascontiguousarray(x[b].T)
        xTq = np.ascontiguousarray(x[b, qb * DQ:(qb + 1) * DQ, :].T)
        qi = qb * DQ + np.arange(DQ)[None, None, :]       # global q index
        kj = (np.arange(NKC)[:, None, None] * KC
              + np.arange(KC)[None, :, None])             # global key index
        mask = (qi >= kj).astype(np.float32)              # [NKC, KC, DQ]
        in_maps.append({
            "xT": xT, "xTq": xTq,
            "wq": wq, "wk": wk, "wv": wv, "wo": wo,
            "bq": bq, "bk": bk, "bv": bv, "bo": bo,
            "onesr": np.ones((1, 128), np.float32),
            "zerosr": np.zeros((1, 512), np.float32),
            "maskT": np.ascontiguousarray(mask.astype(mdt)),
            "mask1T": np.ascontiguousarray((1.0 - mask).astype(mdt)),
        })
    return in_maps


def kernel(x, W_atten, b_atten, W_proj, b_proj):
    nc = build_program(reps=1)
    in_maps = prep_inputs(x, W_atten, b_atten, W_proj, b_proj)
    res = bass_utils.run_bass_kernel_spmd(
        nc, in_maps, core_ids=list(range(N_CORES)))
    out = np.empty((B, S, E), dtype=np.float32)
    for core in range(N_CORES):
        b, qb = core // 4, core % 4
        out[b, qb * DQ:(qb + 1) * DQ, :] = res.results[core]["out"]
    return out



# revision 3
# speedup vs baseline: 1.8909x; 1.8909x over previous
"""Trainium2 Bass kernel for nn_Attention_52982716563627.

Module: qkv = x@W_atten + b_atten; per-head scores s = q k^T (no scaling);
mask applied as w*mask + (1-mask)*1e5; softmax over the HEAD axis (quirk!);
a = p @ v; out = a @ W_proj + b_proj.

Key identity: softmax is over heads at each (i,j). Masked entries (+1e5 for
all 16 heads) softmax to exactly 1/16. Scores are bounded (|s| < ~40), so
max-subtraction can be skipped: set masked scores to 0 -> exp=1 -> uniform
1/16 falls out of the normalization automatically:
    e = exp(s * causal_mask);  Z = sum_h e;  p = e / Z;  a = p @ v
which matches the reference exactly (softmax is shift-invariant).

Sharding: 8 cores = 2 batches x 4 query-blocks of 512 rows (SPMD, one
program; per-core differences enter only via input data: xTq slice + masks).
Each core projects Q/K/V only for its own 512 rows; K^T and V are then
AllGathered (bf16) across the 4 cores of each batch group — no duplicated
K/V projection work.

Layouts (per core):
  kT resident [128p=(h%2)*64+d, 8=h//2, 2048 keys] bf16  (head-major cols)
  qT resident [128, 8, 512 own q rows] bf16
  v in DRAM (AG output) [2048 keys, 1024 cols head-major] bf16
  scores computed transposed: sT[key, q] so AV needs no transposes; K=64
  score matmuls on even/odd partition halves pack 2x via PE 64x128 tiling.
  e tiles [128 keys, 16 h, 256 q] bf16; mask post-exp e<-e*m+(1-m);
  Z = chain-sum over h; p = e * (1/Z) broadcast.
  AV accumulates over 16 key chunks into 4 PSUM banks (4 heads per bank:
  2 on partition halves x 2 on free halves); each bank is one accumulation
  group opened by a zeroing K=1 matmul.
  QKV/proj matmul operands are float32r (1 cyc/row on PE, fp32 storage).

reps>1 python-unrolls the whole body (collectives are not allowed inside
hardware control flow), for repeat-loop timing differencing.
"""

import numpy as np
import ml_dtypes

import concourse.bass as bass
import concourse.bacc as bacc
import concourse.mybir as mybir
import concourse.tile as tile
from concourse import bass_utils

N_CORES = 8
B, S, E = 2, 2048, 1024
H, HD = 16, 64
DQ = 512   # q rows per core
QH = 256   # q tile (half block)
KC = 128   # key chunk
NKC = S // KC  # 16
FP = mybir.dt.float32
FR = mybir.dt.float32r
BF = mybir.dt.bfloat16
AF = mybir.ActivationFunctionType
RG = [[0, 1, 2, 3], [4, 5, 6, 7]]  # batch groups

MQ = FR  # QKV + proj matmul operand dtype


def _bcast_mid(ap, n):
    """View a [128, Q] AP as [128, n, Q] with 0-stride middle dim."""
    return bass.AP(tensor=ap.tensor, offset=ap.offset,
                   ap=[ap.ap[0], [0, n], ap.ap[1]])


def build_program(reps: int = 1):
    nc = bacc.Bacc("TRN2", target_bir_lowering=False, debug=False,
                   num_devices=N_CORES)

    xTq_d = nc.dram_tensor("xTq", [E, DQ], MQ, kind="ExternalInput")
    wq_d = nc.dram_tensor("wq", [E, E], MQ, kind="ExternalInput")
    wk_d = nc.dram_tensor("wk", [E, E], MQ, kind="ExternalInput")
    wv_d = nc.dram_tensor("wv", [E, E], MQ, kind="ExternalInput")
    wo_d = nc.dram_tensor("wo", [E, E], MQ, kind="ExternalInput")
    bq_d = nc.dram_tensor("bq", [128, 8], FP, kind="ExternalInput")
    bk_d = nc.dram_tensor("bk", [128, 8], FP, kind="ExternalInput")
    bv_d = nc.dram_tensor("bv", [1, E], MQ, kind="ExternalInput")
    bo_d = nc.dram_tensor("bo", [1, E], MQ, kind="ExternalInput")
    onesr_d = nc.dram_tensor("onesr", [1, 128], MQ, kind="ExternalInput")
    zerosr_d = nc.dram_tensor("zerosr", [1, 512], MQ, kind="ExternalInput")
    mask_d = nc.dram_tensor("maskT", [NKC, KC, DQ], BF, kind="ExternalInput")
    mask1_d = nc.dram_tensor("mask1T", [NKC, KC, DQ], BF,
                             kind="ExternalInput")
    out_d = nc.dram_tensor("out", [DQ, E], FP, kind="ExternalOutput")

    # collective buffers (HBM; Shared not supported for 4-core groups)
    ktag_in = nc.dram_tensor("ktag_in", [E, DQ], BF, kind="Internal")
    ktag_out = nc.dram_tensor("ktag_out", [4 * E, DQ], BF, kind="Internal")
    vag_in = nc.dram_tensor("vag_in", [DQ, E], BF, kind="Internal")
    vag_out = nc.dram_tensor("vag_out", [S, E], BF, kind="Internal")

    with tile.TileContext(nc) as tc:
        with (
            tc.tile_pool(name="consts", bufs=1) as consts,
            tc.tile_pool(name="kt", bufs=1) as ktp,
            tc.tile_pool(name="qt", bufs=1) as qtp,
        ):
            ones_sb = consts.tile([1, 128], MQ)
            zeros_sb = consts.tile([1, 512], MQ)
            bq_sb = consts.tile([128, 8], FP)
            bk_sb = consts.tile([128, 8], FP)
            bv_sb = consts.tile([1, E], MQ)
            bo_sb = consts.tile([1, E], MQ)
            nc.sync.dma_start(bq_sb[:], bq_d[:])
            nc.sync.dma_start(bk_sb[:], bk_d[:])
            nc.sync.dma_start(ones_sb[:], onesr_d[:])
            nc.sync.dma_start(zeros_sb[:], zerosr_d[:])
            nc.sync.dma_start(bv_sb[:], bv_d[:])
            nc.sync.dma_start(bo_sb[:], bo_d[:])

            kt = ktp.tile([128, 8, S], BF)
            qt = qtp.tile([128, 8, DQ], BF)

            for _rep in range(reps):
                # ---------- Phase A: own qT/kT/v + AllGather K,V ----------
                with (
                    tc.tile_pool(name="xtq", bufs=1) as xtqp,
                    tc.tile_pool(name="wstr", bufs=2) as wstr,
                    tc.tile_pool(name="wvp", bufs=2) as wvp,
                    tc.tile_pool(name="vout", bufs=2) as voutp,
                    tc.tile_pool(name="kout", bufs=2) as koutp,
                    tc.tile_pool(name="pskq", bufs=2, space="PSUM") as pskq,
                    tc.tile_pool(name="psv", bufs=2, space="PSUM") as psv,
                ):
                    xtq = xtqp.tile([128, 8, DQ], MQ)
                    for e in range(8):
                        nc.sync.dma_start(
                            xtq[:, e, :], xTq_d[e * 128:(e + 1) * 128, :])

                    # own kT -> ktag_in (feeds the AG; compute first)
                    for ct in range(8):
                        wct = wstr.tile([128, 8, 128], MQ, tag="wct")
                        nc.sync.dma_start(
                            wct[:],
                            wk_d[:, ct * 128:(ct + 1) * 128].rearrange(
                                "(e p) c -> p e c", p=128))
                        ps = pskq.tile([128, DQ], FP)
                        for e in range(8):
                            nc.tensor.matmul(ps[:], wct[:, e, :],
                                             xtq[:, e, :],
                                             start=(e == 0), stop=(e == 7))
                        ko = koutp.tile([128, DQ], BF, tag="ko")
                        nc.scalar.activation(ko[:], ps[:], AF.Identity,
                                             bias=bk_sb[:, ct:ct + 1])
                        nc.sync.dma_start(
                            ktag_in[ct * 128:(ct + 1) * 128, :], ko[:])

                    # own v (natural [rows, cols]) -> vag_in
                    for cc in range(2):
                        wvcc = wvp.tile([128, 8, 512], MQ, tag="wvcc")
                        nc.sync.dma_start(
                            wvcc[:],
                            wv_d[:, cc * 512:(cc + 1) * 512].rearrange(
                                "(e p) c -> p e c", p=128))
                        for rt in range(4):
                            ps = psv.tile([128, 512], FP)
                            for e in range(8):
                                nc.tensor.matmul(
                                    ps[:],
                                    xtq[:, e, rt * 128:(rt + 1) * 128],
                                    wvcc[:, e, :], start=(e == 0), stop=False)
                            nc.tensor.matmul(ps[:], ones_sb[:1, :],
                                             bv_sb[:1, cc * 512:(cc + 1) * 512],
                                             start=False, stop=True)
                            vo = voutp.tile([128, 512], BF, tag="vo")
                            nc.scalar.activation(vo[:], ps[:], AF.Copy)
                            nc.sync.dma_start(
                                vag_in[rt * 128:(rt + 1) * 128,
                                       cc * 512:(cc + 1) * 512], vo[:])

                    nc.gpsimd.collective_compute(
                        "AllGather", mybir.AluOpType.bypass,
                        replica_groups=RG,
                        ins=[ktag_in[:]], outs=[ktag_out[:]])
                    nc.gpsimd.collective_compute(
                        "AllGather", mybir.AluOpType.bypass,
                        replica_groups=RG,
                        ins=[vag_in[:]], outs=[vag_out[:]])

                    # own qT (overlaps with the AGs)
                    for ct in range(8):
                        wct = wstr.tile([128, 8, 128], MQ, tag="wct")
                        nc.sync.dma_start(
                            wct[:],
                            wq_d[:, ct * 128:(ct + 1) * 128].rearrange(
                                "(e p) c -> p e c", p=128))
                        ps = pskq.tile([128, DQ], FP)
                        for e in range(8):
                            nc.tensor.matmul(ps[:], wct[:, e, :],
                                             xtq[:, e, :],
                                             start=(e == 0), stop=(e == 7))
                        nc.scalar.activation(qt[:, ct, :], ps[:], AF.Identity,
                                             bias=bq_sb[:, ct:ct + 1])

                    # gathered kT -> resident SBUF tile
                    for r in range(4):
                        for e in range(8):
                            nc.sync.dma_start(
                                kt[:, e, r * DQ:(r + 1) * DQ],
                                ktag_out[r * E + e * 128:
                                         r * E + (e + 1) * 128, :])

                # ---------------- Phase B: attention + proj ----------------
                with (
                    tc.tile_pool(name="ebig", bufs=2) as ebp,
                    tc.tile_pool(name="zr", bufs=3) as zrp,
                    tc.tile_pool(name="mstr", bufs=3) as mstr,
                    tc.tile_pool(name="vstr", bufs=3) as vstr,
                    tc.tile_pool(name="at", bufs=2) as atp,
                    tc.tile_pool(name="wop", bufs=2) as wop,
                    tc.tile_pool(name="outp", bufs=2) as outp,
                    tc.tile_pool(name="pss", bufs=3, space="PSUM") as pss,
                    tc.tile_pool(name="psav", bufs=1, space="PSUM") as psav,
                    tc.tile_pool(name="pspj", bufs=1, space="PSUM") as pspj,
                ):
                    for qh in range(2):
                        av = [psav.tile([128, 512], FP, tag=f"av{g}",
                                        name=f"av{g}")
                              for g in range(4)]
                        for g in range(4):
                            nc.tensor.matmul(av[g][:], ones_sb[:1, :],
                                             zeros_sb[:1, :], start=True,
                                             stop=False, skip_group_check=True)
                        for c in range(NKC):
                            mt = mstr.tile([128, QH], BF, tag="mt")
                            nc.sync.dma_start(
                                mt[:], mask_d[c, :, qh * QH:(qh + 1) * QH])
                            m1 = mstr.tile([128, QH], BF, tag="m1")
                            nc.sync.dma_start(
                                m1[:], mask1_d[c, :, qh * QH:(qh + 1) * QH])
                            vch = vstr.tile([128, E], BF, tag="vch")
                            nc.sync.dma_start(
                                vch[:], vag_out[c * 128:(c + 1) * 128, :])
                            eb = ebp.tile([128, H, QH], BF, tag="eb")
                            for h in range(H):
                                po = (h % 2) * 64
                                ps = pss.tile([128, QH], FP, tag="ps")
                                nc.tensor.matmul(
                                    ps[:],
                                    kt[po:po + 64, h // 2,
                                       c * 128:(c + 1) * 128],
                                    qt[po:po + 64, h // 2,
                                       qh * QH:(qh + 1) * QH],
                                    start=True, stop=True)
                                nc.scalar.activation(eb[:, h, :], ps[:],
                                                     AF.Exp)
                            # e <- e*m + (1-m), batched over heads
                            nc.vector.tensor_mul(eb[:], eb[:],
                                                 _bcast_mid(mt[:], H))
                            nc.vector.tensor_add(eb[:], eb[:],
                                                 _bcast_mid(m1[:], H))
                            # Z = sum over heads (chained adds), R = 1/Z
                            z = zrp.tile([128, QH], BF, tag="z")
                            nc.vector.tensor_add(z[:], eb[:, 0, :],
                                                 eb[:, 1, :])
                            for h in range(2, H):
                                nc.vector.tensor_add(z[:], z[:], eb[:, h, :])
                            zf = zrp.tile([128, QH], FP, tag="zf")
                            nc.vector.tensor_copy(zf[:], z[:])
                            r = zrp.tile([128, QH], FP, tag="r")
                            nc.vector.reciprocal_approx_fast(r[:], zf[:])
                            rb = zrp.tile([128, QH], BF, tag="rb")
                            nc.vector.tensor_copy(rb[:], r[:])
                            nc.vector.tensor_mul(eb[:], eb[:],
                                                 _bcast_mid(rb[:], H))
                            for h in range(H):
                                g, g2, po = h // 4, (h // 2) % 2, (h % 2) * 64
                                nc.tensor.matmul(
                                    av[g][po:po + 64, g2 * QH:(g2 + 1) * QH],
                                    vch[:, h * 64:(h + 1) * 64], eb[:, h, :],
                                    start=False, stop=False,
                                    skip_group_check=True)
                        for g in range(4):
                            nc.tensor.matmul(av[g][:], ones_sb[:1, :],
                                             zeros_sb[:1, :], start=False,
                                             stop=True, skip_group_check=True)
                        # aT and projection for this q-half
                        at = atp.tile([128, 8, QH], MQ, tag="at")
                        for j in range(8):
                            nc.scalar.activation(
                                at[:, j, :],
                                av[j // 2][:, (j % 2) * QH:(j % 2 + 1) * QH],
                                AF.Copy)
                        for cc in range(2):
                            wocc = wop.tile([128, 8, 512], MQ, tag="wocc")
                            nc.sync.dma_start(
                                wocc[:],
                                wo_d[:, cc * 512:(cc + 1) * 512].rearrange(
                                    "(e p) c -> p e c", p=128))
                            for qs in range(2):
                                ps = pspj.tile([128, 512], FP, tag="pj")
                                for e in range(8):
                                    nc.tensor.matmul(
                                        ps[:],
                                        at[:, e, qs * 128:(qs + 1) * 128],
                                        wocc[:, e, :],
                                        start=(e == 0), stop=False)
                                nc.tensor.matmul(
                                    ps[:], ones_sb[:1, :],
                                    bo_sb[:1, cc * 512:(cc + 1) * 512],
                                    start=False, stop=True)
                                ot = outp.tile([128, 512], FP, tag="ot")
                                nc.scalar.activation(ot[:], ps[:], AF.Copy)
                                nc.sync.dma_start(
                                    out_d[qh * QH + qs * 128:
                                          qh * QH + (qs + 1) * 128,
                                          cc * 512:(cc + 1) * 512], ot[:])

    nc.compile()
    return nc


def prep_inputs(x, W_atten, b_atten, W_proj, b_proj):
    """Host-side prep: per-core input dicts (numpy)."""
    x = np.asarray(x, dtype=np.float32)
    W3 = np.asarray(W_atten, dtype=np.float32).reshape(E, H, 3, HD)
    b3 = np.asarray(b_atten, dtype=np.float32).reshape(H, 3, HD)
    wq = np.ascontiguousarray(W3[:, :, 0, :].reshape(E, E))
    wk = np.ascontiguousarray(W3[:, :, 1, :].reshape(E, E))
    wv = np.ascontiguousarray(W3[:, :, 2, :].reshape(E, E))
    bq = np.ascontiguousarray(b3[:, 0, :].reshape(E).reshape(8, 128).T)
    bk = np.ascontiguousarray(b3[:, 1, :].reshape(E).reshape(8, 128).T)
    bv = b3[:, 2, :].reshape(1, E).copy()
    wo = np.asarray(W_proj, dtype=np.float32)
    bo = np.asarray(b_proj, dtype=np.float32).reshape(1, E).copy()

    in_maps = []
    for core in range(N_CORES):
        b, qb = core // 4, core % 4
        xTq = np.ascontiguousarray(x[b, qb * DQ:(qb + 1) * DQ, :].T)
        qi = qb * DQ + np.arange(DQ)[None, None, :]       # global q index
        kj = (np.arange(NKC)[:, None, None] * KC
              + np.arange(KC)[None, :, None])             # global key index
        mask = (qi >= kj).astype(np.float32)              # [NKC, KC, DQ]
        in_maps.append({
            "xTq": xTq,
            "wq": wq, "wk": wk, "wv": wv, "wo": wo,
            "bq": bq, "bk": bk, "bv": bv, "bo": bo,
            "onesr": np.ones((1, 128), np.float32),
            "zerosr": np.zeros((1, 512), np.float32),
            "maskT": np.ascontiguousarray(mask.astype(ml_dtypes.bfloat16)),
            "mask1T": np.ascontiguousarray(
                (1.0 - mask).astype(ml_dtypes.bfloat16)),
        })
    return in_maps


def kernel(x, W_atten, b_atten, W_proj, b_proj):
    nc = build_program(reps=1)
    in_maps = prep_inputs(x, W_atten, b_atten, W_proj, b_proj)
    res = bass_utils.run_bass_kernel_spmd(
        nc, in_maps, core_ids=list(range(N_CORES)))
    out = np.empty((B, S, E), dtype=np.float32)
    for core in range(N_CORES):
        b, qb = core // 4, core % 4
        out[b, qb * DQ:(qb + 1) * DQ, :] = res.results[core]["out"]
    return out
